# revision 42
# baseline (speedup 1.0000x reference)
"""Trainium2 Bass kernel for nn_NeuralMemory (B=4, N=1024, D=128, DEPTH=4).

Sharding: 8 cores, core c handles batch b = c//2. Both cores of a pair
compute the store phase (per-token grads summed over all 1024 tokens)
redundantly -- the grad sum is order-invariant, so each core gets its
batch's sequence with its own retrieval half rotated to the front and
retrieves tokens [0:512) of its view. No collectives (a pair AllReduce
has a ~10us floor, worse than the duplicated compute).

v4 design notes:
  - All on-chip tensors are bf16 except f32 PSUM accumulators, so every
    producer writes bf16 directly (no cast chain).
  - Host-side prep (layout/weight-space only, no token-dim compute):
    seq is shipped twice in bf16 -- token-major (s_tmb, for M = S^T G0)
    and feature-major (st = S^T, for the forward); the [d,d] weights
    ship pre-transposed/pre-scaled/pre-composed in bf16 packs (Wk@w0
    alone in a 32KB first DMA so layer 0 starts right after S^T lands;
    wq@Wk^T so x0 and pt are independent matmuls off S^T; w^T's,
    +-(2/D) scales, identity) plus a small f32 pack for the
    u_i = w_i + dW_i adds.
  - Dependency tracking is tile-granular, so every per-half-written or
    per-half-read tensor is split into separate tiles (a_i, g_i, sp_i,
    c_i, H_i, px_i, r_i, o_tm halves) -- otherwise write-after-read
    false deps serialize the two pipelines.
  - forward Silu / backward Derivative_silu read the f32 H PSUM banks
    directly; H0..H2 live in six single-bank tiles whose banks are
    reused by the backward c tiles, then the retrieval px tiles.
  - token-major copies for the dW contraction run on the DMA XBAR
    (dma_start_transpose, SBUF->SBUF bf16, ~450ns per [128,512] half on
    otherwise-idle DMA engines); only g0 (which gates M on the critical
    tail) keeps the lower-latency PE-transpose + DVE-evict path.
  - backward runs tile-1-first layer-major; M = S^T G0 accumulates in
    its own PSUM bank (so m_r never waits the dW group's late xbar
    inputs); dW3/dW2/dW1 share a second accumulation group feeding the
    u_i adds. Retrieval is X1 = X0 w0 + (X0 Wk^T) M with the X0 w0 term
    pre-accumulated, so only the tiny M eviction sits on the critical
    tail; the output ships in the device's token-chunk layout and the
    host reassembles.
  - ACT-table discipline: all Silus, then all Derivative_silus, then a
    dummy Silu reload during the dW phase.
"""

import numpy as np
import ml_dtypes

import concourse.bass as bass
import concourse.mybir as mybir
import concourse.tile as tile
from concourse import bacc
from concourse.bass import ts
from concourse.bass_utils import run_bass_kernel_spmd

B, N, D = 4, 1024, 128
NCORES = 8
NT = 512            # tokens retrieved per core (half a batch)
TT = 512            # store-phase token tile
NTI = N // TT
NCHUNK = N // 128
RH = 256            # retrieval sub-tile

# bf16 weight packs:
#  wpbu (urgent): w0eff=Wk@w0 | w1 | w2 | w3s=(2/D)w3 | wv_r=-(2/D)Wv
#                 | wq | wkq_t=wq@Wk^T
#  wpbr (rest):   w1^T | w2^T | w3^T | w0 | ident

f32 = mybir.dt.float32
bf16 = mybir.dt.bfloat16

AF = mybir.ActivationFunctionType
ALU = mybir.AluOpType


def _build_program(reps=1):
    nc = bacc.Bacc(
        "TRN2",
        target_bir_lowering=False,
        debug=False,
        enable_asserts=False,
        num_devices=NCORES,
    )

    st_dr = nc.dram_tensor("st", [128, N], bf16, kind="ExternalInput").ap()
    stm_dr = nc.dram_tensor("s_tmb", [128, N], bf16, kind="ExternalInput").ap()
    we_dr = nc.dram_tensor("w0eff", [D, D], bf16, kind="ExternalInput").ap()
    wbu_dr = nc.dram_tensor("wpbu", [D, 6 * D], bf16, kind="ExternalInput").ap()
    wbr_dr = nc.dram_tensor("wpbr", [D, 5 * D], bf16, kind="ExternalInput").ap()
    wf_dr = nc.dram_tensor("wpf", [D, 3 * D], f32, kind="ExternalInput").ap()
    out_dr = nc.dram_tensor("out", [128, NT // 128, D], bf16, kind="ExternalOutput").ap()

    with tile.TileContext(nc) as tc:
        for _ in range(reps):
            _emit(tc, st_dr, stm_dr, we_dr, wbu_dr, wbr_dr, wf_dr, out_dr)

    nc.compile()
    return nc


def _emit(tc, st_dr, stm_dr, we_dr, wbu_dr, wbr_dr, wf_dr, out_dr):
    nc = tc.nc
    from contextlib import ExitStack

    from concourse.tile_rust import add_dep_helper as _dep  # type: ignore

    with ExitStack() as ctx:
        consts = ctx.enter_context(tc.tile_pool(name="consts", bufs=1))
        big = ctx.enter_context(tc.tile_pool(name="big", bufs=1))
        # PSUM: ha0,ha1,hb0,hb1,hc0,hc1 (1 bank each) + st(2) = 8 banks
        pp = ctx.enter_context(tc.tile_pool(name="pp", bufs=1, space="PSUM"))

        def pbank(tag, name, shape=None, dt=f32):
            return pp.tile(shape or [128, TT], dt, tag=tag, bufs=1, name=name)

        def pstage(name, w=512, dt=f32):
            return pp.tile([128, w], dt, tag="stg", bufs=2, name=name)

        # ---- DMAs, ordered by need (all HWDGE on the sync queue);
        # w0eff ships alone (32KB) so layer 0 starts right after S^T ----
        w0eff_t = consts.tile([D, D], bf16, tag="w0eff")
        wpbu = consts.tile([D, 6 * D], bf16, tag="wpbu")
        wpbr = consts.tile([D, 5 * D], bf16, tag="wpbr")
        sttf = big.tile([128, N], bf16, tag="stt")     # S^T feature-major
        stt = [sttf[:, ts(t, TT)] for t in range(NTI)]
        s_tmb = big.tile([128, NCHUNK, 128], bf16, tag="s_tmb")  # token-major
        wpf = consts.tile([D, 3 * D], f32, tag="wpf")
        nc.sync.dma_start(sttf[:], st_dr)
        nc.sync.dma_start(w0eff_t[:], we_dr)
        nc.sync.dma_start(wpbu[:], wbu_dr)
        nc.sync.dma_start(wpbr[:], wbr_dr)
        nc.sync.dma_start(
            s_tmb[:], stm_dr.rearrange("p (c d) -> p c d", d=128)
        )
        nc.sync.dma_start(wpf[:], wf_dr)

        w0eff = w0eff_t[:]
        w1b = wpbu[:, ts(0, D)]
        w2b = wpbu[:, ts(1, D)]
        w3s = wpbu[:, ts(2, D)]
        wv_r = wpbu[:, ts(3, D)]
        wqb = wpbu[:, ts(4, D)]
        wkq_t = wpbu[:, ts(5, D)]                      # wq @ Wk^T
        wt = [wpbr[:, ts(i, D)] for i in range(3)]     # w1^T,w2^T,w3^T
        w0b = wpbr[:, ts(3, D)]
        ident_b = wpbr[:, ts(4, D)]
        w_f = [wpf[:, ts(i, D)] for i in range(3)]     # w1,w2,w3 f32

        # tiny scratch silu pulls the first ACT table load off the
        # critical path (runs during the DMAs)
        scr = consts.tile([128, 1], f32, tag="scr")
        scr2 = consts.tile([128, 1], f32, tag="scr2")
        nc.gpsimd.memset(scr[:], 0.0)
        nc.scalar.activation(scr2[:], scr[:], AF.Silu)

        # PE warm-up: start the HAM clock window early so real matmuls
        # hit full clock by ~3us
        wupa = consts.tile([128, 128], f32, tag="wupa")
        nc.gpsimd.memset(wupa[:], 0.0)
        wupp = pstage("wupp")
        for _ in range(4):
            nc.tensor.matmul(
                wupp[:, 0:128], wupa[:], wupa[:],
                skip_group_check=True,
            )

        # per-half persistent SBUF tensors (feature-major, bf16)
        def halves(pfx):
            return [
                big.tile([128, TT], bf16, name=f"{pfx}{t}", tag=f"{pfx}{t}")
                for t in range(NTI)
            ]

        a1 = halves("a1")
        a2 = halves("a2")
        a3 = halves("a3")
        sp0 = halves("sp0")
        sp1 = halves("sp1")
        sp2 = halves("sp2")
        g0 = halves("g0")
        g1 = halves("g1")
        g2 = halves("g2")
        g3 = halves("g3")   # (2/D)(H3 - V)

        # ---- x0 = wq^T S^T and pt = (wq Wk^T)^T S^T: two independent
        # matmuls straight off S^T (host pre-composed wkq_t = wq @ Wk^T),
        # evicted on DVE long before the backward muls need it ----
        x0 = big.tile([128, NT], bf16, tag="x0")
        px = pstage("p_x0")
        nc.tensor.matmul(px[:], wqb, stt[0])
        nc.vector.tensor_copy(x0[:], px[:])
        pt = big.tile([128, NT], bf16, tag="pt")
        px = pstage("p_pt")
        nc.tensor.matmul(px[:], wkq_t, stt[0])
        nc.vector.tensor_copy(pt[:], px[:])

        silu_insts = []
        dsilu_insts = []

        # ---- forward: six single-bank H tiles; silu -> bf16 halves ----
        hb = {}
        for li, tag in ((0, "ha"), (1, "hb"), (2, "hc")):
            for t in range(NTI):
                hb[li, t] = pbank(f"{tag}{t}", f"h{li}_{t}")
        for t in range(NTI):
            nc.tensor.matmul(hb[0, t][:], w0eff, stt[t])
            silu_insts.append(nc.scalar.activation(a1[t][:], hb[0, t][:], AF.Silu))
            nc.tensor.matmul(hb[1, t][:], w1b, a1[t][:])
            silu_insts.append(nc.scalar.activation(a2[t][:], hb[1, t][:], AF.Silu))
            nc.tensor.matmul(hb[2, t][:], w2b, a2[t][:])
            silu_insts.append(nc.scalar.activation(a3[t][:], hb[2, t][:], AF.Silu))
            # H3 - V accumulated in one stage bank (wv_r is negated+scaled)
            h3 = pstage(f"h3_{t}")
            nc.tensor.matmul(h3[:], wv_r, stt[t], start=True, stop=False)
            nc.tensor.matmul(h3[:], w3s, a3[t][:], start=False, stop=True)
            nc.vector.tensor_copy(g3[t][:], h3[:])

        # ---- dsilu after all silus (one table switch), consumption order --
        for spd, li in ((sp2, 2), (sp1, 1), (sp0, 0)):
            for t in (1, 0):
                di = nc.scalar.activation(
                    spd[t][:], hb[li, t][:], AF.Derivative_silu
                )
                dsilu_insts.append(di)
        for di in dsilu_insts:
            _dep(di.ins, silu_insts[-1].ins, sync=False, reason="act-table order")

        # ---- backward: tile-1 first (it gates M); c tiles reuse the H
        # banks (freed in dsilu order, which matches mul consumption) ----
        cb = {}
        for li, tag in ((2, "hc"), (1, "hb"), (0, "ha")):
            for t in (1, 0):
                cb[li, t] = pbank(f"{tag}{t}", f"c{li}_{t}")
        last_cmm = None
        for li, gin, gout, spd in (
            (2, g3, g2, sp2), (1, g2, g1, sp1), (0, g1, g0, sp0)
        ):
            for t in (1, 0):
                last_cmm = nc.tensor.matmul(cb[li, t][:], wt[li], gin[t][:])
                nc.vector.tensor_mul(gout[t][:], cb[li, t][:], spd[t][:])

        # reload the silu table during the dW phase, off the tail
        scr3 = consts.tile([128, 1], f32, tag="scr3")
        dummy = nc.scalar.activation(scr3[:], scr[:], AF.Silu)
        _dep(dummy.ins, dsilu_insts[-1].ins, sync=False, reason="act-table order")

        # ---- token-major copies: XBAR for a1..a3,g3..g1; PE path for g0 ----
        a_tm = [None] + [
            big.tile([128, NCHUNK, 128], bf16, name=f"atm{i}", tag=f"atm{i}")
            for i in (1, 2, 3)
        ]
        g_tm = [
            big.tile([128, NCHUNK, 128], bf16, name=f"gtm{i}", tag=f"gtm{i}")
            for i in range(4)
        ]
        # g1's t1 half dispatches first (its mul finishes first) so the
        # head-of-line SP dispatch queue matches readiness order
        for src, dst, horder in (
            (a1, a_tm[1], (0, 1)), (a2, a_tm[2], (0, 1)), (a3, a_tm[3], (0, 1)),
            (g3, g_tm[3], (0, 1)), (g2, g_tm[2], (0, 1)), (g1, g_tm[1], (1, 0)),
        ):
            for h in horder:
                nc.sync.dma_start_transpose(dst[:, 4 * h : 4 * h + 4], src[h][:])

        # g0: PE transposes + DVE half evicts (lowest latency on the
        # tail); separate stage tiles per half so the h0 transposes don't
        # wait on the h1 eviction (tile-granular WAR). Both g0 muls are
        # emitted before the evicts so DVE drains the muls first.
        pg0 = [pstage(f"p_g0{h}", w=512, dt=bf16) for h in range(NTI)]
        last_pg0 = None
        for h in (1, 0):
            for j in range(4):
                last_pg0 = nc.tensor.matmul(
                    pg0[h][:, ts(j, 128)], g0[h][:, ts(j, 128)], ident_b,
                    is_transpose=True,
                )
        # h1 evict on ACT (idle during the dW phase); h0 on DVE -- they
        # drain in parallel instead of serializing on DVE
        nc.scalar.activation(
            g_tm[0][:, 4:8].rearrange("p c d -> p (c d)"), pg0[1][:], AF.Copy
        )
        nc.vector.tensor_copy(
            g_tm[0][:, 0:4], pg0[0][:].rearrange("p (c d) -> p c d", d=128)
        )

        # ---- M = S^T G0 in its OWN bank/group so m_r never waits the
        # late dW1 xbar round-trip ----
        macc = pstage("macc", w=128)
        m_stop = None
        for k, c in enumerate((4, 5, 6, 7, 0, 1, 2, 3)):
            m_stop = nc.tensor.matmul(
                macc[:, 0:128],
                s_tmb[:, c],
                g_tm[0][:, c],
                start=(k == 0),
                stop=(c == 3),
            )
        m_r = big.tile([128, 128], bf16, tag="m_r")
        # ACT is idle here; DVE is still draining the g0 evicts
        nc.scalar.activation(m_r[:], macc[:, 0:128], AF.Copy)

        # ---- dW1 in its OWN bank/group: u1 feeds retrieval layer 2 and
        # must not wait behind dW3/dW2's group stop ----
        dw1acc = pstage("dw1acc", w=128)
        dw1_stop = None
        dw1_first = None
        for c in range(NCHUNK):
            dw1_stop = nc.tensor.matmul(
                dw1acc[:, 0:128],
                a_tm[1][:, c],
                g_tm[1][:, c],
                start=(c == 0),
                stop=(c == NCHUNK - 1),
            )
            if c == 0:
                dw1_first = dw1_stop
                _dep(dw1_first.ins, last_cmm.ins, sync=False,
                     reason="PE order: backward before dW")
                _dep(dw1_first.ins, last_pg0.ins, sync=False,
                     reason="PE order: g0 transposes before dW")

        # ---- dW3/dW2 in one PSUM accumulation group ----
        acc = pbank("hc1", "dwacc", shape=[128, 2, 128])
        dw_stop = None
        dw_first = None
        for k, (i, slot) in enumerate(((3, 0), (2, 1))):
            for c in range(NCHUNK):
                dw_stop = nc.tensor.matmul(
                    acc[:, slot],
                    a_tm[i][:, c],
                    g_tm[i][:, c],
                    start=(k == 0 and c == 0),
                    stop=(slot == 1 and c == NCHUNK - 1),
                )
                if dw_first is None:
                    dw_first = dw_stop
                    _dep(dw_first.ins, last_cmm.ins, sync=False,
                         reason="PE order: backward before dW")
                    _dep(dw_first.ins, last_pg0.ins, sync=False,
                         reason="PE order: g0 transposes before dW")

        u = [None]
        ut = big.tile([D, D], bf16, name="u1", tag="u1")
        ai = nc.vector.tensor_add(ut[:], dw1acc[:, 0:128], w_f[0])
        _dep(ai.ins, dw1_stop.ins, sync=True, reason="dw1 bank group")
        u.append(ut)
        for slot, i in ((1, 1), (0, 2)):
            ut = big.tile([D, D], bf16, name=f"u{i + 1}", tag=f"u{i + 1}")
            ai = nc.vector.tensor_add(ut[:], acc[:, slot], w_f[i])
            # same-bank safety: no reads before the group's stop matmul
            _dep(ai.ins, dw_stop.ins, sync=True, reason="acc bank group")
            u.append(ut)
        # u[1]=w1+dW1, u[2]=w2+dW2, u[3]=w3+dW3

        # ---- retrieval: X1 = X0 @ w0 + P @ M, then layers 2..4 ------------
        # per-half tiles throughout so the two half-pipelines don't
        # serialize on tile-granular deps
        r1, r2, r3 = [], [], []
        for h in range(NTI):
            r1.append(big.tile([128, RH], bf16, name=f"r1h{h}", tag=f"r1h{h}"))
            r2.append(big.tile([128, RH], bf16, name=f"r2h{h}", tag=f"r2h{h}"))
            r3.append(big.tile([128, RH], bf16, name=f"r3h{h}", tag=f"r3h{h}"))

        nh = NT // RH
        px1 = [pbank(f"ha{hh}", f"px1_{hh}", shape=[128, RH]) for hh in range(nh)]
        for hh in range(nh):
            # term 1 (X0 @ w0) has no M dependency -- runs during the dW phase
            t1mm = nc.tensor.matmul(
                px1[hh][:], w0b, x0[:, ts(hh, RH)], start=True, stop=False
            )
            _dep(t1mm.ins, last_pg0.ins, sync=False,
                 reason="PE order: g0 transposes before px1 term1")
        for hh in range(nh):
            smm = nc.tensor.matmul(
                px1[hh][:], m_r[:], pt[:, ts(hh, RH)], start=False, stop=True
            )
            # the dW blocks stall mid-group on late XBAR inputs; keep the
            # retrieval-start matmuls ahead of them in the static PE order
            _dep(dw_first.ins, smm.ins, sync=False,
                 reason="PE order: X1 stop before dW3/dW2")
            _dep(dw1_first.ins, smm.ins, sync=False,
                 reason="PE order: X1 stop before dW1")
            nc.scalar.activation(r1[hh][:], px1[hh][:], AF.Silu)
        px2 = [pbank(f"hb{hh}", f"px2_{hh}", shape=[128, RH]) for hh in range(nh)]
        for hh in range(nh):
            pmm = nc.tensor.matmul(px2[hh][:], u[1][:], r1[hh][:])
            nc.scalar.activation(r2[hh][:], px2[hh][:], AF.Silu)
        px3 = [
            pbank("hc0", "px3_0", shape=[128, RH]),
            pstage("px3_1", w=RH),
        ]
        for hh in range(nh):
            nc.tensor.matmul(px3[hh][:], u[2][:], r2[hh][:])
            nc.scalar.activation(r3[hh][:], px3[hh][:], AF.Silu)
        out_r = out_dr  # [p, c, d]: token c*128+p, contiguous per partition
        for hh in range(nh):
            po = pstage(f"po{hh}", w=RH)
            pov = po[:].rearrange("p (c d) -> p c d", d=128)
            for j in range(RH // 128):
                nc.tensor.matmul(
                    pov[:, j], r3[hh][:, ts(j, 128)], u[3][:],
                    start=(j == 0), stop=(j == RH // 128 - 1),
                )
            o_tm = big.tile([128, 2, 128], bf16, name=f"o_tm{hh}", tag=f"o_tm{hh}")
            nc.vector.tensor_copy(o_tm[:], pov[:])
            nc.sync.dma_start(out_r[:, 2 * hh : 2 * hh + 2], o_tm[:])


_CACHE = {}


def _get_nc():
    if "nc" not in _CACHE:
        _CACHE["nc"] = _build_program()
    return _CACHE["nc"]


def _bf(x):
    return np.ascontiguousarray(x.astype(ml_dtypes.bfloat16))


def _prep_weights(w0, w1, w2, w3, wq, wkv):
    """Host-side weight-space prep (layout, transposes, scales, composes)."""
    w0, w1, w2, w3, wq, wkv = (
        np.asarray(x, np.float32) for x in (w0, w1, w2, w3, wq, wkv)
    )
    wk, wv = wkv[:, :D], wkv[:, D:]
    ident = np.eye(D, dtype=np.float32)
    w0eff = wk @ w0
    wpbu = np.concatenate(
        [
            w1, w2,
            (2.0 / D) * w3,     # w3s
            (-2.0 / D) * wv,    # wv_r
            wq,                 # wqb
            wq @ wk.T,          # wkq_t: pt = (wq Wk^T)^T S^T
        ],
        axis=1,
    )
    wpbr = np.concatenate([w1.T, w2.T, w3.T, w0, ident], axis=1)
    wpf = np.ascontiguousarray(np.concatenate([w1, w2, w3], axis=1))
    return _bf(w0eff), _bf(wpbu), _bf(wpbr), wpf


def kernel(seq, w0, w1, w2, w3, wq, wkv):
    nc = _get_nc()
    seq = np.asarray(seq, np.float32)
    w0eff, wpbu, wpbr, wpf = _prep_weights(w0, w1, w2, w3, wq, wkv)

    in_maps = []
    for c in range(NCORES):
        b, h = c // 2, c % 2
        if h == 0:
            s = seq[b]
        else:
            # rotate: retrieval half first; grad sum is order-invariant
            s = np.concatenate([seq[b, NT:], seq[b, :NT]], axis=0)
        sb = s.astype(ml_dtypes.bfloat16)
        # token-major [128, c, d] flattened: partition p, token c*128+p
        stm = np.ascontiguousarray(
            sb.reshape(NCHUNK, 128, D).transpose(1, 0, 2).reshape(128, N)
        )
        in_maps.append(
            {
                "st": np.ascontiguousarray(sb.T),
                "s_tmb": stm,
                "w0eff": w0eff,
                "wpbu": wpbu,
                "wpbr": wpbr,
                "wpf": wpf,
            }
        )

    res = run_bass_kernel_spmd(nc, in_maps, core_ids=list(range(NCORES)))
    _CACHE["last_results"] = res

    out = np.empty((B, N, D), np.float32)
    for c in range(NCORES):
        b, h = c // 2, c % 2
        # device layout [p, chunk, d] -> tokens (chunk*128+p, d)
        ob = res.results[c]["out"].astype(np.float32)
        out[b, h * NT : (h + 1) * NT] = ob.transpose(1, 0, 2).reshape(NT, D)
    return out


# revision 46
# speedup vs baseline: 1.0133x; 1.0133x over previous
"""Trainium2 Bass kernel for nn_NeuralMemory (B=4, N=1024, D=128, DEPTH=4).

Sharding: 8 cores, core c handles batch b = c//2. Both cores of a pair
compute the store phase (per-token grads summed over all 1024 tokens)
redundantly -- the grad sum is order-invariant, so each core gets its
batch's sequence with its own retrieval half rotated to the front and
retrieves tokens [0:512) of its view. No collectives (a pair AllReduce
has a ~10us floor, worse than the duplicated compute).

v4 design notes:
  - All on-chip tensors are bf16 except f32 PSUM accumulators, so every
    producer writes bf16 directly (no cast chain).
  - Host-side prep (layout/weight-space only, no token-dim compute):
    seq is shipped twice in bf16 -- token-major (s_tmb, for M = S^T G0)
    and feature-major (st = S^T, for the forward); the [d,d] weights
    ship pre-transposed/pre-scaled/pre-composed in bf16 packs (Wk@w0
    alone in a 32KB first DMA so layer 0 starts right after S^T lands;
    wq@Wk^T so x0 and pt are independent matmuls off S^T; w^T's,
    +-(2/D) scales, identity) plus a small f32 pack for the
    u_i = w_i + dW_i adds.
  - Dependency tracking is tile-granular, so every per-half-written or
    per-half-read tensor is split into separate tiles (a_i, g_i, sp_i,
    c_i, H_i, px_i, r_i, o_tm halves) -- otherwise write-after-read
    false deps serialize the two pipelines.
  - forward Silu / backward Derivative_silu read the f32 H PSUM banks
    directly; H0..H2 live in six single-bank tiles whose banks are
    reused by the backward c tiles, then the retrieval px tiles.
  - token-major copies for the dW contraction run on the DMA XBAR
    (dma_start_transpose, SBUF->SBUF bf16, ~450ns per [128,512] half on
    otherwise-idle DMA engines); only g0 (which gates M on the critical
    tail) keeps the lower-latency PE-transpose + DVE-evict path.
  - backward runs tile-1-first layer-major; M = S^T G0 accumulates in
    its own PSUM bank (so m_r never waits the dW group's late xbar
    inputs); dW3/dW2/dW1 share a second accumulation group feeding the
    u_i adds. Retrieval is X1 = X0 w0 + (X0 Wk^T) M with the X0 w0 term
    pre-accumulated, so only the tiny M eviction sits on the critical
    tail; the output ships in the device's token-chunk layout and the
    host reassembles.
  - ACT-table discipline: all Silus, then all Derivative_silus, then a
    dummy Silu reload during the dW phase.
"""

import numpy as np
import ml_dtypes

import concourse.bass as bass
import concourse.mybir as mybir
import concourse.tile as tile
from concourse import bacc
from concourse.bass import ts
from concourse.bass_utils import run_bass_kernel_spmd

B, N, D = 4, 1024, 128
NCORES = 8
NT = 512            # tokens retrieved per core (half a batch)
TT = 512            # store-phase token tile
NTI = N // TT
NCHUNK = N // 128
RH = 256            # retrieval sub-tile

# bf16 weight packs:
#  wpbu (urgent): w0eff=Wk@w0 | w1 | w2 | w3s=(2/D)w3 | wv_r=-(2/D)Wv
#                 | wq | wkq_t=wq@Wk^T
#  wpbr (rest):   w1^T | w2^T | w3^T | w0 | ident

f32 = mybir.dt.float32
bf16 = mybir.dt.bfloat16

AF = mybir.ActivationFunctionType
ALU = mybir.AluOpType


def _build_program(reps=1):
    nc = bacc.Bacc(
        "TRN2",
        target_bir_lowering=False,
        debug=False,
        enable_asserts=False,
        num_devices=NCORES,
    )

    st_dr = nc.dram_tensor("st", [128, N], bf16, kind="ExternalInput").ap()
    stm_dr = nc.dram_tensor("s_tmb", [128, N], bf16, kind="ExternalInput").ap()
    we_dr = nc.dram_tensor("w0eff", [D, D], bf16, kind="ExternalInput").ap()
    wbu_dr = nc.dram_tensor("wpbu", [D, 6 * D], bf16, kind="ExternalInput").ap()
    wbr_dr = nc.dram_tensor("wpbr", [D, 5 * D], bf16, kind="ExternalInput").ap()
    wf_dr = nc.dram_tensor("wpf", [D, 3 * D], f32, kind="ExternalInput").ap()
    out_dr = nc.dram_tensor("out", [128, NT // 128, D], bf16, kind="ExternalOutput").ap()

    with tile.TileContext(nc) as tc:
        for _ in range(reps):
            _emit(tc, st_dr, stm_dr, we_dr, wbu_dr, wbr_dr, wf_dr, out_dr)

    nc.compile()
    return nc


def _emit(tc, st_dr, stm_dr, we_dr, wbu_dr, wbr_dr, wf_dr, out_dr):
    nc = tc.nc
    from contextlib import ExitStack

    from concourse.tile_rust import add_dep_helper as _dep  # type: ignore

    with ExitStack() as ctx:
        consts = ctx.enter_context(tc.tile_pool(name="consts", bufs=1))
        big = ctx.enter_context(tc.tile_pool(name="big", bufs=1))
        # PSUM: ha0,ha1,hb0,hb1,hc0,hc1 (1 bank each) + st(2) = 8 banks
        pp = ctx.enter_context(tc.tile_pool(name="pp", bufs=1, space="PSUM"))

        def pbank(tag, name, shape=None, dt=f32):
            return pp.tile(shape or [128, TT], dt, tag=tag, bufs=1, name=name)

        def pstage(name, w=512, dt=f32):
            return pp.tile([128, w], dt, tag="stg", bufs=2, name=name)

        # ---- DMAs, ordered by need (all HWDGE on the sync queue);
        # w0eff ships alone (32KB) so layer 0 starts right after S^T ----
        w0eff_t = consts.tile([D, D], bf16, tag="w0eff")
        wpbu = consts.tile([D, 6 * D], bf16, tag="wpbu")
        wpbr = consts.tile([D, 5 * D], bf16, tag="wpbr")
        sttf = big.tile([128, N], bf16, tag="stt")     # S^T feature-major
        stt = [sttf[:, ts(t, TT)] for t in range(NTI)]
        s_tmb = big.tile([128, NCHUNK, 128], bf16, tag="s_tmb")  # token-major
        wpf = consts.tile([D, 3 * D], f32, tag="wpf")
        nc.sync.dma_start(sttf[:], st_dr)
        nc.sync.dma_start(w0eff_t[:], we_dr)
        nc.sync.dma_start(wpbu[:], wbu_dr)
        nc.sync.dma_start(wpbr[:], wbr_dr)
        nc.sync.dma_start(
            s_tmb[:], stm_dr.rearrange("p (c d) -> p c d", d=128)
        )
        nc.sync.dma_start(wpf[:], wf_dr)

        w0eff = w0eff_t[:]
        w1b = wpbu[:, ts(0, D)]
        w2b = wpbu[:, ts(1, D)]
        w3s = wpbu[:, ts(2, D)]
        wv_r = wpbu[:, ts(3, D)]
        wqb = wpbu[:, ts(4, D)]
        wkq_t = wpbu[:, ts(5, D)]                      # wq @ Wk^T
        wt = [wpbr[:, ts(i, D)] for i in range(3)]     # w1^T,w2^T,w3^T
        w0b = wpbr[:, ts(3, D)]
        ident_b = wpbr[:, ts(4, D)]
        w_f = [wpf[:, ts(i, D)] for i in range(3)]     # w1,w2,w3 f32

        # tiny scratch silu pulls the first ACT table load off the
        # critical path (runs during the DMAs)
        scr = consts.tile([128, 1], f32, tag="scr")
        scr2 = consts.tile([128, 1], f32, tag="scr2")
        nc.gpsimd.memset(scr[:], 0.0)
        nc.scalar.activation(scr2[:], scr[:], AF.Silu)

        # PE warm-up: start the HAM clock window early so real matmuls
        # hit full clock by ~3us
        wupa = consts.tile([128, 128], f32, tag="wupa")
        nc.gpsimd.memset(wupa[:], 0.0)
        wupp = pstage("wupp")
        for _ in range(4):
            nc.tensor.matmul(
                wupp[:, 0:128], wupa[:], wupa[:],
                skip_group_check=True,
            )

        # per-half persistent SBUF tensors (feature-major, bf16)
        def halves(pfx):
            return [
                big.tile([128, TT], bf16, name=f"{pfx}{t}", tag=f"{pfx}{t}")
                for t in range(NTI)
            ]

        a1 = halves("a1")
        a2 = halves("a2")
        a3 = halves("a3")
        sp0 = halves("sp0")
        sp1 = halves("sp1")
        sp2 = halves("sp2")
        g0 = halves("g0")
        g1 = halves("g1")
        g2 = halves("g2")
        g3 = halves("g3")   # (2/D)(H3 - V)

        # ---- x0 = wq^T S^T and pt = (wq Wk^T)^T S^T: two independent
        # matmuls straight off S^T (host pre-composed wkq_t = wq @ Wk^T),
        # evicted on DVE long before the backward muls need it ----
        x0 = big.tile([128, NT], bf16, tag="x0")
        px = pstage("p_x0")
        nc.tensor.matmul(px[:], wqb, stt[0])
        nc.vector.tensor_copy(x0[:], px[:])
        pt = big.tile([128, NT], bf16, tag="pt")
        px = pstage("p_pt")
        nc.tensor.matmul(px[:], wkq_t, stt[0])
        nc.vector.tensor_copy(pt[:], px[:])

        silu_insts = []
        dsilu_insts = []

        # ---- forward: six single-bank H tiles; silu -> bf16 halves ----
        hb = {}
        for li, tag in ((0, "ha"), (1, "hb"), (2, "hc")):
            for t in range(NTI):
                hb[li, t] = pbank(f"{tag}{t}", f"h{li}_{t}")
        for t in range(NTI):
            nc.tensor.matmul(hb[0, t][:], w0eff, stt[t])
            silu_insts.append(nc.scalar.activation(a1[t][:], hb[0, t][:], AF.Silu))
            nc.tensor.matmul(hb[1, t][:], w1b, a1[t][:])
            silu_insts.append(nc.scalar.activation(a2[t][:], hb[1, t][:], AF.Silu))
            nc.tensor.matmul(hb[2, t][:], w2b, a2[t][:])
            silu_insts.append(nc.scalar.activation(a3[t][:], hb[2, t][:], AF.Silu))
            # H3 - V accumulated in one stage bank (wv_r is negated+scaled)
            h3 = pstage(f"h3_{t}")
            nc.tensor.matmul(h3[:], wv_r, stt[t], start=True, stop=False)
            nc.tensor.matmul(h3[:], w3s, a3[t][:], start=False, stop=True)
            nc.vector.tensor_copy(g3[t][:], h3[:])

        # ---- dsilu after all silus (one table switch), consumption order --
        for spd, li in ((sp2, 2), (sp1, 1), (sp0, 0)):
            for t in (1, 0):
                di = nc.scalar.activation(
                    spd[t][:], hb[li, t][:], AF.Derivative_silu
                )
                dsilu_insts.append(di)
        for di in dsilu_insts:
            _dep(di.ins, silu_insts[-1].ins, sync=False, reason="act-table order")

        # ---- backward: tile-1 first (it gates M); c tiles reuse the H
        # banks (freed in dsilu order, which matches mul consumption) ----
        cb = {}
        for li, tag in ((2, "hc"), (1, "hb"), (0, "ha")):
            for t in (1, 0):
                cb[li, t] = pbank(f"{tag}{t}", f"c{li}_{t}")
        last_cmm = None
        for li, gin, gout, spd in (
            (2, g3, g2, sp2), (1, g2, g1, sp1), (0, g1, g0, sp0)
        ):
            for t in (1, 0):
                last_cmm = nc.tensor.matmul(cb[li, t][:], wt[li], gin[t][:])
                nc.vector.tensor_mul(gout[t][:], cb[li, t][:], spd[t][:])

        # reload the silu table during the dW phase, off the tail
        scr3 = consts.tile([128, 1], f32, tag="scr3")
        dummy = nc.scalar.activation(scr3[:], scr[:], AF.Silu)
        _dep(dummy.ins, dsilu_insts[-1].ins, sync=False, reason="act-table order")

        # ---- token-major copies: XBAR for a1..a3,g3..g1; PE path for g0 ----
        a_tm = [None] + [
            big.tile([128, NCHUNK, 128], bf16, name=f"atm{i}", tag=f"atm{i}")
            for i in (1, 2, 3)
        ]
        g_tm = [
            big.tile([128, NCHUNK, 128], bf16, name=f"gtm{i}", tag=f"gtm{i}")
            for i in range(4)
        ]
        # g1's t1 half dispatches first (its mul finishes first) so the
        # head-of-line SP dispatch queue matches readiness order
        for src, dst, horder in (
            (a1, a_tm[1], (0, 1)), (a2, a_tm[2], (0, 1)), (a3, a_tm[3], (0, 1)),
            (g3, g_tm[3], (0, 1)), (g2, g_tm[2], (0, 1)),
        ):
            for h in horder:
                nc.sync.dma_start_transpose(dst[:, 4 * h : 4 * h + 4], src[h][:])

        # g1 feeds dW1 -> u1 -> retrieval layer 2 and the XBAR pipe
        # (dispatch+gen+latency+transfer ~2.5us) is too slow for it: use
        # PE transposes into the idle hb banks + DVE evicts slotted into
        # the gaps around the g0 eviction. t1 half here; t0 half below,
        # after the g0 section, so its evict queues behind pg0's.
        pg1 = [None, None]
        pg1[1] = pbank("hb1", "pg1_1", shape=[128, TT], dt=bf16)
        for j in range(4):
            nc.tensor.matmul(
                pg1[1][:, ts(j, 128)], g1[1][:, ts(j, 128)], ident_b,
                is_transpose=True,
            )
        nc.vector.tensor_copy(
            g_tm[1][:, 4:8], pg1[1][:].rearrange("p (c d) -> p c d", d=128)
        )

        # g0: PE transposes + DVE half evicts (lowest latency on the
        # tail); separate stage tiles per half so the h0 transposes don't
        # wait on the h1 eviction (tile-granular WAR). Both g0 muls are
        # emitted before the evicts so DVE drains the muls first.
        pg0 = [pstage(f"p_g0{h}", w=512, dt=bf16) for h in range(NTI)]
        last_pg0 = None
        for h in (1, 0):
            for j in range(4):
                last_pg0 = nc.tensor.matmul(
                    pg0[h][:, ts(j, 128)], g0[h][:, ts(j, 128)], ident_b,
                    is_transpose=True,
                )
        # h1 evict on ACT (idle during the dW phase); h0 on DVE -- they
        # drain in parallel instead of serializing on DVE
        nc.scalar.activation(
            g_tm[0][:, 4:8].rearrange("p c d -> p (c d)"), pg0[1][:], AF.Copy
        )
        nc.vector.tensor_copy(
            g_tm[0][:, 0:4], pg0[0][:].rearrange("p (c d) -> p c d", d=128)
        )
        pg1[0] = pbank("hb0", "pg1_0", shape=[128, TT], dt=bf16)
        for j in range(4):
            nc.tensor.matmul(
                pg1[0][:, ts(j, 128)], g1[0][:, ts(j, 128)], ident_b,
                is_transpose=True,
            )
        nc.vector.tensor_copy(
            g_tm[1][:, 0:4], pg1[0][:].rearrange("p (c d) -> p c d", d=128)
        )

        # ---- M = S^T G0 in its OWN bank/group so m_r never waits the
        # late dW1 xbar round-trip ----
        macc = pstage("macc", w=128)
        m_stop = None
        for k, c in enumerate((4, 5, 6, 7, 0, 1, 2, 3)):
            m_stop = nc.tensor.matmul(
                macc[:, 0:128],
                s_tmb[:, c],
                g_tm[0][:, c],
                start=(k == 0),
                stop=(c == 3),
            )
        m_r = big.tile([128, 128], bf16, tag="m_r")
        # ACT is idle here; DVE is still draining the g0 evicts
        nc.scalar.activation(m_r[:], macc[:, 0:128], AF.Copy)

        # ---- dW1 in its OWN bank/group: u1 feeds retrieval layer 2 and
        # must not wait behind dW3/dW2's group stop ----
        dw1acc = pstage("dw1acc", w=128)
        dw1_stop = None
        dw1_first = None
        for k, c in enumerate((4, 5, 6, 7, 0, 1, 2, 3)):
            dw1_stop = nc.tensor.matmul(
                dw1acc[:, 0:128],
                a_tm[1][:, c],
                g_tm[1][:, c],
                start=(k == 0),
                stop=(c == 3),
            )
            if k == 0:
                dw1_first = dw1_stop
                _dep(dw1_first.ins, last_cmm.ins, sync=False,
                     reason="PE order: backward before dW")
                _dep(dw1_first.ins, last_pg0.ins, sync=False,
                     reason="PE order: g0 transposes before dW")

        # ---- dW2 in hb1 (free between pg1_1's evict and px2h1): u2
        # lands right after X1stop instead of waiting dW3's group ----
        dw2acc = pbank("hb1", "dw2acc", shape=[128, 128])
        dw2_stop = None
        for c in range(NCHUNK):
            dw2_stop = nc.tensor.matmul(
                dw2acc[:],
                a_tm[2][:, c],
                g_tm[2][:, c],
                start=(c == 0),
                stop=(c == NCHUNK - 1),
            )
            if c == 0:
                _dep(dw2_stop.ins, last_cmm.ins, sync=False,
                     reason="PE order: backward before dW")
                _dep(dw2_stop.ins, last_pg0.ins, sync=False,
                     reason="PE order: g0 transposes before dW")

        # ---- dW3 group (only feeds u3, needed last) ----
        acc = pbank("hc1", "dwacc", shape=[128, 128])
        dw_stop = None
        dw_first = None
        for c in range(NCHUNK):
            dw_stop = nc.tensor.matmul(
                acc[:],
                a_tm[3][:, c],
                g_tm[3][:, c],
                start=(c == 0),
                stop=(c == NCHUNK - 1),
            )
            if dw_first is None:
                dw_first = dw_stop
                _dep(dw_first.ins, last_cmm.ins, sync=False,
                     reason="PE order: backward before dW")
                _dep(dw_first.ins, last_pg0.ins, sync=False,
                     reason="PE order: g0 transposes before dW")

        u = [None]
        for nm, accb, stop_i, wf in (
            ("u1", dw1acc[:, 0:128], dw1_stop, w_f[0]),
            ("u2", dw2acc[:], dw2_stop, w_f[1]),
            ("u3", acc[:], dw_stop, w_f[2]),
        ):
            ut = big.tile([D, D], bf16, name=nm, tag=nm)
            ai = nc.vector.tensor_add(ut[:], accb, wf)
            # same-bank safety: no reads before the group's stop matmul
            _dep(ai.ins, stop_i.ins, sync=True, reason=f"{nm} bank group")
            u.append(ut)
        # u[1]=w1+dW1, u[2]=w2+dW2, u[3]=w3+dW3

        # ---- retrieval: X1 = X0 @ w0 + P @ M, then layers 2..4 ------------
        # per-half tiles throughout so the two half-pipelines don't
        # serialize on tile-granular deps
        r1, r2, r3 = [], [], []
        for h in range(NTI):
            r1.append(big.tile([128, RH], bf16, name=f"r1h{h}", tag=f"r1h{h}"))
            r2.append(big.tile([128, RH], bf16, name=f"r2h{h}", tag=f"r2h{h}"))
            r3.append(big.tile([128, RH], bf16, name=f"r3h{h}", tag=f"r3h{h}"))

        nh = NT // RH
        px1 = [pbank(f"ha{hh}", f"px1_{hh}", shape=[128, RH]) for hh in range(nh)]
        for hh in range(nh):
            # term 1 (X0 @ w0) has no M dependency -- runs during the dW phase
            t1mm = nc.tensor.matmul(
                px1[hh][:], w0b, x0[:, ts(hh, RH)], start=True, stop=False
            )
            _dep(t1mm.ins, last_pg0.ins, sync=False,
                 reason="PE order: g0 transposes before px1 term1")
        for hh in range(nh):
            smm = nc.tensor.matmul(
                px1[hh][:], m_r[:], pt[:, ts(hh, RH)], start=False, stop=True
            )
            # the dW blocks stall mid-group on late XBAR inputs; keep the
            # retrieval-start matmuls ahead of them in the static PE order
            _dep(dw_first.ins, smm.ins, sync=False,
                 reason="PE order: X1 stop before dW3")
            _dep(dw2_stop.ins, smm.ins, sync=False,
                 reason="PE order: X1 stop before dW2 tail")
            nc.scalar.activation(r1[hh][:], px1[hh][:], AF.Silu)
        px2 = [pbank(f"hb{hh}", f"px2_{hh}", shape=[128, RH]) for hh in range(nh)]
        for hh in range(nh):
            pmm = nc.tensor.matmul(px2[hh][:], u[1][:], r1[hh][:])
            _dep(dw_first.ins, pmm.ins, sync=False,
                 reason="PE order: px2 before dW3")
            nc.scalar.activation(r2[hh][:], px2[hh][:], AF.Silu)
        px3 = [
            pbank("hc0", "px3_0", shape=[128, RH]),
            pstage("px3_1", w=RH),
        ]
        for hh in range(nh):
            nc.tensor.matmul(px3[hh][:], u[2][:], r2[hh][:])
            nc.scalar.activation(r3[hh][:], px3[hh][:], AF.Silu)
        out_r = out_dr  # [p, c, d]: token c*128+p, contiguous per partition
        for hh in range(nh):
            po = pstage(f"po{hh}", w=RH)
            pov = po[:].rearrange("p (c d) -> p c d", d=128)
            for j in range(RH // 128):
                nc.tensor.matmul(
                    pov[:, j], r3[hh][:, ts(j, 128)], u[3][:],
                    start=(j == 0), stop=(j == RH // 128 - 1),
                )
            o_tm = big.tile([128, 2, 128], bf16, name=f"o_tm{hh}", tag=f"o_tm{hh}")
            nc.vector.tensor_copy(o_tm[:], pov[:])
            nc.sync.dma_start(out_r[:, 2 * hh : 2 * hh + 2], o_tm[:])


_CACHE = {}


def _get_nc():
    if "nc" not in _CACHE:
        _CACHE["nc"] = _build_program()
    return _CACHE["nc"]


def _bf(x):
    return np.ascontiguousarray(x.astype(ml_dtypes.bfloat16))


def _prep_weights(w0, w1, w2, w3, wq, wkv):
    """Host-side weight-space prep (layout, transposes, scales, composes)."""
    w0, w1, w2, w3, wq, wkv = (
        np.asarray(x, np.float32) for x in (w0, w1, w2, w3, wq, wkv)
    )
    wk, wv = wkv[:, :D], wkv[:, D:]
    ident = np.eye(D, dtype=np.float32)
    w0eff = wk @ w0
    wpbu = np.concatenate(
        [
            w1, w2,
            (2.0 / D) * w3,     # w3s
            (-2.0 / D) * wv,    # wv_r
            wq,                 # wqb
            wq @ wk.T,          # wkq_t: pt = (wq Wk^T)^T S^T
        ],
        axis=1,
    )
    wpbr = np.concatenate([w1.T, w2.T, w3.T, w0, ident], axis=1)
    wpf = np.ascontiguousarray(np.concatenate([w1, w2, w3], axis=1))
    return _bf(w0eff), _bf(wpbu), _bf(wpbr), wpf


def kernel(seq, w0, w1, w2, w3, wq, wkv):
    nc = _get_nc()
    seq = np.asarray(seq, np.float32)
    w0eff, wpbu, wpbr, wpf = _prep_weights(w0, w1, w2, w3, wq, wkv)

    in_maps = []
    for c in range(NCORES):
        b, h = c // 2, c % 2
        if h == 0:
            s = seq[b]
        else:
            # rotate: retrieval half first; grad sum is order-invariant
            s = np.concatenate([seq[b, NT:], seq[b, :NT]], axis=0)
        sb = s.astype(ml_dtypes.bfloat16)
        # token-major [128, c, d] flattened: partition p, token c*128+p
        stm = np.ascontiguousarray(
            sb.reshape(NCHUNK, 128, D).transpose(1, 0, 2).reshape(128, N)
        )
        in_maps.append(
            {
                "st": np.ascontiguousarray(sb.T),
                "s_tmb": stm,
                "w0eff": w0eff,
                "wpbu": wpbu,
                "wpbr": wpbr,
                "wpf": wpf,
            }
        )

    res = run_bass_kernel_spmd(nc, in_maps, core_ids=list(range(NCORES)))
    _CACHE["last_results"] = res

    out = np.empty((B, N, D), np.float32)
    for c in range(NCORES):
        b, h = c // 2, c % 2
        # device layout [p, chunk, d] -> tokens (chunk*128+p, d)
        ob = res.results[c]["out"].astype(np.float32)
        out[b, h * NT : (h + 1) * NT] = ob.transpose(1, 0, 2).reshape(NT, D)
    return out


# revision 49
# speedup vs baseline: 1.0211x; 1.0077x over previous
"""Trainium2 Bass kernel for nn_NeuralMemory (B=4, N=1024, D=128, DEPTH=4).

Sharding: 8 cores, core c handles batch b = c//2. Both cores of a pair
compute the store phase (per-token grads summed over all 1024 tokens)
redundantly -- the grad sum is order-invariant, so each core gets its
batch's sequence with its own retrieval half rotated to the front and
retrieves tokens [0:512) of its view. No collectives (a pair AllReduce
has a ~10us floor, worse than the duplicated compute).

v4 design notes:
  - All on-chip tensors are bf16 except f32 PSUM accumulators, so every
    producer writes bf16 directly (no cast chain).
  - Host-side prep (layout/weight-space only, no token-dim compute):
    seq is shipped twice in bf16 -- token-major (s_tmb, for M = S^T G0)
    and feature-major (st = S^T, for the forward); the [d,d] weights
    ship pre-transposed/pre-scaled/pre-composed in bf16 packs (Wk@w0
    alone in a 32KB first DMA so layer 0 starts right after S^T lands;
    wq@Wk^T so x0 and pt are independent matmuls off S^T; w^T's,
    +-(2/D) scales, identity) plus a small f32 pack for the
    u_i = w_i + dW_i adds.
  - Dependency tracking is tile-granular, so every per-half-written or
    per-half-read tensor is split into separate tiles (a_i, g_i, sp_i,
    c_i, H_i, px_i, r_i, o_tm halves) -- otherwise write-after-read
    false deps serialize the two pipelines.
  - forward Silu / backward Derivative_silu read the f32 H PSUM banks
    directly; H0..H2 live in six single-bank tiles whose banks are
    reused by the backward c tiles, then the retrieval px tiles.
  - token-major copies for the dW contraction run on the DMA XBAR
    (dma_start_transpose, SBUF->SBUF bf16, ~450ns per [128,512] half on
    otherwise-idle DMA engines); only g0 (which gates M on the critical
    tail) keeps the lower-latency PE-transpose + DVE-evict path.
  - backward runs tile-1-first layer-major; M = S^T G0 accumulates in
    its own PSUM bank (so m_r never waits the dW group's late xbar
    inputs); dW3/dW2/dW1 share a second accumulation group feeding the
    u_i adds. Retrieval is X1 = X0 w0 + (X0 Wk^T) M with the X0 w0 term
    pre-accumulated, so only the tiny M eviction sits on the critical
    tail; the output ships in the device's token-chunk layout and the
    host reassembles.
  - ACT-table discipline: all Silus, then all Derivative_silus, then a
    dummy Silu reload during the dW phase.
"""

import numpy as np
import ml_dtypes

import concourse.bass as bass
import concourse.mybir as mybir
import concourse.tile as tile
from concourse import bacc
from concourse.bass import ts
from concourse.bass_utils import run_bass_kernel_spmd

B, N, D = 4, 1024, 128
NCORES = 8
NT = 512            # tokens retrieved per core (half a batch)
TT = 512            # store-phase token tile
NTI = N // TT
NCHUNK = N // 128
RH = 256            # retrieval sub-tile

# bf16 weight packs:
#  wpbu (urgent): w0eff=Wk@w0 | w1 | w2 | w3s=(2/D)w3 | wv_r=-(2/D)Wv
#                 | wq | wkq_t=wq@Wk^T
#  wpbr (rest):   w1^T | w2^T | w3^T | w0 | ident

f32 = mybir.dt.float32
bf16 = mybir.dt.bfloat16

AF = mybir.ActivationFunctionType
ALU = mybir.AluOpType


def _build_program(reps=1):
    nc = bacc.Bacc(
        "TRN2",
        target_bir_lowering=False,
        debug=False,
        enable_asserts=False,
        num_devices=NCORES,
    )

    st_dr = nc.dram_tensor("st", [128, N], bf16, kind="ExternalInput").ap()
    stm_dr = nc.dram_tensor("s_tmb", [128, N], bf16, kind="ExternalInput").ap()
    we_dr = nc.dram_tensor("w0eff", [D, D], bf16, kind="ExternalInput").ap()
    wbu_dr = nc.dram_tensor("wpbu", [D, 6 * D], bf16, kind="ExternalInput").ap()
    wbr_dr = nc.dram_tensor("wpbr", [D, 5 * D], bf16, kind="ExternalInput").ap()
    wf_dr = nc.dram_tensor("wpf", [D, 3 * D], f32, kind="ExternalInput").ap()
    out_dr = nc.dram_tensor("out", [128, NT // 128, D], bf16, kind="ExternalOutput").ap()

    with tile.TileContext(nc) as tc:
        for _ in range(reps):
            _emit(tc, st_dr, stm_dr, we_dr, wbu_dr, wbr_dr, wf_dr, out_dr)

    nc.compile()
    return nc


def _emit(tc, st_dr, stm_dr, we_dr, wbu_dr, wbr_dr, wf_dr, out_dr):
    nc = tc.nc
    from contextlib import ExitStack

    from concourse.tile_rust import add_dep_helper as _dep  # type: ignore

    with ExitStack() as ctx:
        consts = ctx.enter_context(tc.tile_pool(name="consts", bufs=1))
        big = ctx.enter_context(tc.tile_pool(name="big", bufs=1))
        # PSUM: ha0,ha1,hb0,hb1,hc0,hc1 (1 bank each) + st(2) = 8 banks
        pp = ctx.enter_context(tc.tile_pool(name="pp", bufs=1, space="PSUM"))

        def pbank(tag, name, shape=None, dt=f32):
            return pp.tile(shape or [128, TT], dt, tag=tag, bufs=1, name=name)

        def pstage(name, w=512, dt=f32):
            return pp.tile([128, w], dt, tag="stg", bufs=2, name=name)

        # ---- DMAs, ordered by need (all HWDGE on the sync queue);
        # w0eff ships alone (32KB) so layer 0 starts right after S^T ----
        w0eff_t = consts.tile([D, D], bf16, tag="w0eff")
        wpbu = consts.tile([D, 6 * D], bf16, tag="wpbu")
        wpbr = consts.tile([D, 5 * D], bf16, tag="wpbr")
        sttf = big.tile([128, N], bf16, tag="stt")     # S^T feature-major
        stt = [sttf[:, ts(t, TT)] for t in range(NTI)]
        s_tmb = big.tile([128, NCHUNK, 128], bf16, tag="s_tmb")  # token-major
        wpf = consts.tile([D, 3 * D], f32, tag="wpf")
        nc.sync.dma_start(sttf[:], st_dr)
        nc.sync.dma_start(w0eff_t[:], we_dr)
        nc.sync.dma_start(wpbu[:], wbu_dr)
        nc.sync.dma_start(wpbr[:], wbr_dr)
        nc.sync.dma_start(
            s_tmb[:], stm_dr.rearrange("p (c d) -> p c d", d=128)
        )
        nc.sync.dma_start(wpf[:], wf_dr)

        w0eff = w0eff_t[:]
        w1b = wpbu[:, ts(0, D)]
        w2b = wpbu[:, ts(1, D)]
        w3s = wpbu[:, ts(2, D)]
        wv_r = wpbu[:, ts(3, D)]
        wqb = wpbu[:, ts(4, D)]
        wkq_t = wpbu[:, ts(5, D)]                      # wq @ Wk^T
        wt = [wpbr[:, ts(i, D)] for i in range(3)]     # w1^T,w2^T,w3^T
        w0b = wpbr[:, ts(3, D)]
        ident_b = wpbr[:, ts(4, D)]
        w_f = [wpf[:, ts(i, D)] for i in range(3)]     # w1,w2,w3 f32

        # tiny scratch silu pulls the first ACT table load off the
        # critical path (runs during the DMAs)
        scr = consts.tile([128, 1], f32, tag="scr")
        scr2 = consts.tile([128, 1], f32, tag="scr2")
        nc.gpsimd.memset(scr[:], 0.0)
        nc.scalar.activation(scr2[:], scr[:], AF.Silu)

        # PE warm-up: start the HAM clock window early so real matmuls
        # hit full clock by ~3us
        wupa = consts.tile([128, 128], f32, tag="wupa")
        nc.gpsimd.memset(wupa[:], 0.0)
        wupp = pstage("wupp")
        for _ in range(4):
            nc.tensor.matmul(
                wupp[:, 0:128], wupa[:], wupa[:],
                skip_group_check=True,
            )

        # per-half persistent SBUF tensors (feature-major, bf16)
        def halves(pfx):
            return [
                big.tile([128, TT], bf16, name=f"{pfx}{t}", tag=f"{pfx}{t}")
                for t in range(NTI)
            ]

        a1 = halves("a1")
        a2 = halves("a2")
        a3 = halves("a3")
        sp0 = halves("sp0")
        sp1 = halves("sp1")
        sp2 = halves("sp2")
        g0 = halves("g0")
        g1 = halves("g1")
        g2 = halves("g2")
        g3 = halves("g3")   # (2/D)(H3 - V)

        # ---- x0 = wq^T S^T and pt = (wq Wk^T)^T S^T: two independent
        # matmuls straight off S^T (host pre-composed wkq_t = wq @ Wk^T),
        # evicted on DVE long before the backward muls need it ----
        x0 = big.tile([128, NT], bf16, tag="x0")
        px = pstage("p_x0")
        nc.tensor.matmul(px[:], wqb, stt[0])
        nc.vector.tensor_copy(x0[:], px[:])
        pt = big.tile([128, NT], bf16, tag="pt")
        px = pstage("p_pt")
        nc.tensor.matmul(px[:], wkq_t, stt[0])
        nc.vector.tensor_copy(pt[:], px[:])

        silu_insts = []
        dsilu_insts = []

        # ---- forward: six single-bank H tiles; silu -> bf16 halves ----
        hb = {}
        for li, tag in ((0, "ha"), (1, "hb"), (2, "hc")):
            for t in range(NTI):
                hb[li, t] = pbank(f"{tag}{t}", f"h{li}_{t}")
        for t in range(NTI):
            nc.tensor.matmul(hb[0, t][:], w0eff, stt[t])
            silu_insts.append(nc.scalar.activation(a1[t][:], hb[0, t][:], AF.Silu))
            nc.tensor.matmul(hb[1, t][:], w1b, a1[t][:])
            silu_insts.append(nc.scalar.activation(a2[t][:], hb[1, t][:], AF.Silu))
            nc.tensor.matmul(hb[2, t][:], w2b, a2[t][:])
            silu_insts.append(nc.scalar.activation(a3[t][:], hb[2, t][:], AF.Silu))
            # H3 - V accumulated in one stage bank (wv_r is negated+scaled)
            h3 = pstage(f"h3_{t}")
            nc.tensor.matmul(h3[:], wv_r, stt[t], start=True, stop=False)
            nc.tensor.matmul(h3[:], w3s, a3[t][:], start=False, stop=True)
            nc.vector.tensor_copy(g3[t][:], h3[:])

        # ---- dsilu after all silus (one table switch), consumption order --
        for spd, li in ((sp2, 2), (sp1, 1), (sp0, 0)):
            for t in (1, 0):
                di = nc.scalar.activation(
                    spd[t][:], hb[li, t][:], AF.Derivative_silu
                )
                dsilu_insts.append(di)
        for di in dsilu_insts:
            _dep(di.ins, silu_insts[-1].ins, sync=False, reason="act-table order")

        # ---- backward: tile-1 first (it gates M); c tiles reuse the H
        # banks (freed in dsilu order, which matches mul consumption) ----
        cb = {}
        for li, tag in ((2, "hc"), (1, "hb"), (0, "ha")):
            for t in (1, 0):
                cb[li, t] = pbank(f"{tag}{t}", f"c{li}_{t}")
        last_cmm = None
        for li, gin, gout, spd in (
            (2, g3, g2, sp2), (1, g2, g1, sp1), (0, g1, g0, sp0)
        ):
            for t in (1, 0):
                last_cmm = nc.tensor.matmul(cb[li, t][:], wt[li], gin[t][:])
                nc.vector.tensor_mul(gout[t][:], cb[li, t][:], spd[t][:])

        # reload the silu table during the dW phase, off the tail
        scr3 = consts.tile([128, 1], f32, tag="scr3")
        dummy = nc.scalar.activation(scr3[:], scr[:], AF.Silu)
        _dep(dummy.ins, dsilu_insts[-1].ins, sync=False, reason="act-table order")

        # ---- token-major copies: XBAR for a1..a3,g3..g1; PE path for g0 ----
        a_tm = [None] + [
            big.tile([128, NCHUNK, 128], bf16, name=f"atm{i}", tag=f"atm{i}")
            for i in (1, 2, 3)
        ]
        g_tm = [
            big.tile([128, NCHUNK, 128], bf16, name=f"gtm{i}", tag=f"gtm{i}")
            for i in range(4)
        ]
        # g1's t1 half dispatches first (its mul finishes first) so the
        # head-of-line SP dispatch queue matches readiness order
        for src, dst, horder in (
            (a1, a_tm[1], (0, 1)), (a2, a_tm[2], (0, 1)), (a3, a_tm[3], (0, 1)),
            (g3, g_tm[3], (0, 1)), (g2, g_tm[2], (0, 1)),
        ):
            for h in horder:
                nc.sync.dma_start_transpose(dst[:, 4 * h : 4 * h + 4], src[h][:])

        # g1 feeds dW1 -> u1 -> retrieval layer 2 and the XBAR pipe
        # (dispatch+gen+latency+transfer ~2.5us) is too slow for it: use
        # PE transposes into the idle hb banks + DVE evicts slotted into
        # the gaps around the g0 eviction. t1 half here; t0 half below,
        # after the g0 section, so its evict queues behind pg0's.
        pg1 = [None, None]
        pg1[1] = pbank("hb1", "pg1_1", shape=[128, TT], dt=bf16)
        for j in range(4):
            nc.tensor.matmul(
                pg1[1][:, ts(j, 128)], g1[1][:, ts(j, 128)], ident_b,
                is_transpose=True,
            )

        # g0: PE transposes + DVE half evicts (lowest latency on the
        # tail); separate stage tiles per half so the h0 transposes don't
        # wait on the h1 eviction (tile-granular WAR). Both g0 muls are
        # emitted before the evicts so DVE drains the muls first.
        pg0 = [pstage(f"p_g0{h}", w=512, dt=bf16) for h in range(NTI)]
        last_pg0 = None
        for h in (1, 0):
            for j in range(4):
                last_pg0 = nc.tensor.matmul(
                    pg0[h][:, ts(j, 128)], g0[h][:, ts(j, 128)], ident_b,
                    is_transpose=True,
                )
        # h1 evict on ACT (idle during the dW phase); h0 on DVE -- they
        # drain in parallel instead of serializing on DVE
        nc.scalar.activation(
            g_tm[0][:, 4:8].rearrange("p c d -> p (c d)"), pg0[1][:], AF.Copy
        )
        ev_g0h0 = nc.vector.tensor_copy(
            g_tm[0][:, 0:4], pg0[0][:].rearrange("p (c d) -> p c d", d=128)
        )
        pg1[0] = pbank("hb0", "pg1_0", shape=[128, TT], dt=bf16)
        for j in range(4):
            nc.tensor.matmul(
                pg1[0][:, ts(j, 128)], g1[0][:, ts(j, 128)], ident_b,
                is_transpose=True,
            )
        # g1's t1-half evict slots into the DVE gap before the g0 h0 evict
        # is ready; the t0-half must NOT jump ahead of it (g0-h0 gates
        # M -> m_r -> the whole tail)
        nc.vector.tensor_copy(
            g_tm[1][:, 4:8], pg1[1][:].rearrange("p (c d) -> p c d", d=128)
        )
        ev = nc.vector.tensor_copy(
            g_tm[1][:, 0:4], pg1[0][:].rearrange("p (c d) -> p c d", d=128)
        )
        _dep(ev.ins, ev_g0h0.ins, sync=False, reason="DVE order: g0 evict first")

        # ---- M = S^T G0 in its OWN bank/group so m_r never waits the
        # late dW1 xbar round-trip ----
        macc = pstage("macc", w=128)
        m_stop = None
        for k, c in enumerate((4, 5, 6, 7, 0, 1, 2, 3)):
            m_stop = nc.tensor.matmul(
                macc[:, 0:128],
                s_tmb[:, c],
                g_tm[0][:, c],
                start=(k == 0),
                stop=(c == 3),
            )
        m_r = big.tile([128, 128], bf16, tag="m_r")
        # ACT is idle here; DVE is still draining the g0 evicts
        nc.scalar.activation(m_r[:], macc[:, 0:128], AF.Copy)

        # ---- dW1 in its OWN bank/group: u1 feeds retrieval layer 2 and
        # must not wait behind dW3/dW2's group stop ----
        dw1acc = pstage("dw1acc", w=128)
        dw1_stop = None
        dw1_first = None
        for k, c in enumerate((4, 5, 6, 7, 0, 1, 2, 3)):
            dw1_stop = nc.tensor.matmul(
                dw1acc[:, 0:128],
                a_tm[1][:, c],
                g_tm[1][:, c],
                start=(k == 0),
                stop=(c == 3),
            )
            if k == 0:
                dw1_first = dw1_stop
                _dep(dw1_first.ins, last_cmm.ins, sync=False,
                     reason="PE order: backward before dW")
                _dep(dw1_first.ins, last_pg0.ins, sync=False,
                     reason="PE order: g0 transposes before dW")

        # ---- dW2 in hb1 (free between pg1_1's evict and px2h1): u2
        # lands right after X1stop instead of waiting dW3's group ----
        dw2acc = pbank("hb1", "dw2acc", shape=[128, 128])
        dw2_stop = None
        for c in range(NCHUNK):
            dw2_stop = nc.tensor.matmul(
                dw2acc[:],
                a_tm[2][:, c],
                g_tm[2][:, c],
                start=(c == 0),
                stop=(c == NCHUNK - 1),
            )
            if c == 0:
                _dep(dw2_stop.ins, last_cmm.ins, sync=False,
                     reason="PE order: backward before dW")
                _dep(dw2_stop.ins, last_pg0.ins, sync=False,
                     reason="PE order: g0 transposes before dW")

        # ---- dW3 group (only feeds u3, needed last) ----
        acc = pbank("hc1", "dwacc", shape=[128, 128])
        dw_stop = None
        dw_first = None
        for c in range(NCHUNK):
            dw_stop = nc.tensor.matmul(
                acc[:],
                a_tm[3][:, c],
                g_tm[3][:, c],
                start=(c == 0),
                stop=(c == NCHUNK - 1),
            )
            if dw_first is None:
                dw_first = dw_stop
                _dep(dw_first.ins, last_cmm.ins, sync=False,
                     reason="PE order: backward before dW")
                _dep(dw_first.ins, last_pg0.ins, sync=False,
                     reason="PE order: g0 transposes before dW")

        u = [None]
        for nm, accb, stop_i, wf in (
            ("u1", dw1acc[:, 0:128], dw1_stop, w_f[0]),
            ("u2", dw2acc[:], dw2_stop, w_f[1]),
            ("u3", acc[:], dw_stop, w_f[2]),
        ):
            ut = big.tile([D, D], bf16, name=nm, tag=nm)
            ai = nc.vector.tensor_add(ut[:], accb, wf)
            # same-bank safety: no reads before the group's stop matmul
            _dep(ai.ins, stop_i.ins, sync=True, reason=f"{nm} bank group")
            u.append(ut)
        # u[1]=w1+dW1, u[2]=w2+dW2, u[3]=w3+dW3

        # ---- retrieval: X1 = X0 @ w0 + P @ M, then layers 2..4 ------------
        # per-half tiles throughout so the two half-pipelines don't
        # serialize on tile-granular deps
        r1, r2, r3 = [], [], []
        for h in range(NTI):
            r1.append(big.tile([128, RH], bf16, name=f"r1h{h}", tag=f"r1h{h}"))
            r2.append(big.tile([128, RH], bf16, name=f"r2h{h}", tag=f"r2h{h}"))
            r3.append(big.tile([128, RH], bf16, name=f"r3h{h}", tag=f"r3h{h}"))

        nh = NT // RH
        px1 = [pbank(f"ha{hh}", f"px1_{hh}", shape=[128, RH]) for hh in range(nh)]
        for hh in range(nh):
            # term 1 (X0 @ w0) has no M dependency -- runs during the dW phase
            t1mm = nc.tensor.matmul(
                px1[hh][:], w0b, x0[:, ts(hh, RH)], start=True, stop=False
            )
            _dep(t1mm.ins, last_pg0.ins, sync=False,
                 reason="PE order: g0 transposes before px1 term1")
        for hh in range(nh):
            smm = nc.tensor.matmul(
                px1[hh][:], m_r[:], pt[:, ts(hh, RH)], start=False, stop=True
            )
            # the dW blocks stall mid-group on late XBAR inputs; keep the
            # retrieval-start matmuls ahead of them in the static PE order
            _dep(dw_first.ins, smm.ins, sync=False,
                 reason="PE order: X1 stop before dW3")
            _dep(dw2_stop.ins, smm.ins, sync=False,
                 reason="PE order: X1 stop before dW2 tail")
            nc.scalar.activation(r1[hh][:], px1[hh][:], AF.Silu)
        px2 = [pbank(f"hb{hh}", f"px2_{hh}", shape=[128, RH]) for hh in range(nh)]
        for hh in range(nh):
            pmm = nc.tensor.matmul(px2[hh][:], u[1][:], r1[hh][:])
            _dep(dw_first.ins, pmm.ins, sync=False,
                 reason="PE order: px2 before dW3")
            nc.scalar.activation(r2[hh][:], px2[hh][:], AF.Silu)
        px3 = [
            pbank("hc0", "px3_0", shape=[128, RH]),
            pstage("px3_1", w=RH),
        ]
        for hh in range(nh):
            nc.tensor.matmul(px3[hh][:], u[2][:], r2[hh][:])
            nc.scalar.activation(r3[hh][:], px3[hh][:], AF.Silu)
        out_r = out_dr  # [p, c, d]: token c*128+p, contiguous per partition
        for hh in range(nh):
            po = pstage(f"po{hh}", w=RH)
            pov = po[:].rearrange("p (c d) -> p c d", d=128)
            for j in range(RH // 128):
                nc.tensor.matmul(
                    pov[:, j], r3[hh][:, ts(j, 128)], u[3][:],
                    start=(j == 0), stop=(j == RH // 128 - 1),
                )
            o_tm = big.tile([128, 2, 128], bf16, name=f"o_tm{hh}", tag=f"o_tm{hh}")
            nc.vector.tensor_copy(o_tm[:], pov[:])
            nc.sync.dma_start(out_r[:, 2 * hh : 2 * hh + 2], o_tm[:])


_CACHE = {}


def _get_nc():
    if "nc" not in _CACHE:
        _CACHE["nc"] = _build_program()
    return _CACHE["nc"]


def _bf(x):
    return np.ascontiguousarray(x.astype(ml_dtypes.bfloat16))


def _prep_weights(w0, w1, w2, w3, wq, wkv):
    """Host-side weight-space prep (layout, transposes, scales, composes)."""
    w0, w1, w2, w3, wq, wkv = (
        np.asarray(x, np.float32) for x in (w0, w1, w2, w3, wq, wkv)
    )
    wk, wv = wkv[:, :D], wkv[:, D:]
    ident = np.eye(D, dtype=np.float32)
    w0eff = wk @ w0
    wpbu = np.concatenate(
        [
            w1, w2,
            (2.0 / D) * w3,     # w3s
            (-2.0 / D) * wv,    # wv_r
            wq,                 # wqb
            wq @ wk.T,          # wkq_t: pt = (wq Wk^T)^T S^T
        ],
        axis=1,
    )
    wpbr = np.concatenate([w1.T, w2.T, w3.T, w0, ident], axis=1)
    wpf = np.ascontiguousarray(np.concatenate([w1, w2, w3], axis=1))
    return _bf(w0eff), _bf(wpbu), _bf(wpbr), wpf


def kernel(seq, w0, w1, w2, w3, wq, wkv):
    nc = _get_nc()
    seq = np.asarray(seq, np.float32)
    w0eff, wpbu, wpbr, wpf = _prep_weights(w0, w1, w2, w3, wq, wkv)

    in_maps = []
    for c in range(NCORES):
        b, h = c // 2, c % 2
        if h == 0:
            s = seq[b]
        else:
            # rotate: retrieval half first; grad sum is order-invariant
            s = np.concatenate([seq[b, NT:], seq[b, :NT]], axis=0)
        sb = s.astype(ml_dtypes.bfloat16)
        # token-major [128, c, d] flattened: partition p, token c*128+p
        stm = np.ascontiguousarray(
            sb.reshape(NCHUNK, 128, D).transpose(1, 0, 2).reshape(128, N)
        )
        in_maps.append(
            {
                "st": np.ascontiguousarray(sb.T),
                "s_tmb": stm,
                "w0eff": w0eff,
                "wpbu": wpbu,
                "wpbr": wpbr,
                "wpf": wpf,
            }
        )

    res = run_bass_kernel_spmd(nc, in_maps, core_ids=list(range(NCORES)))
    _CACHE["last_results"] = res

    out = np.empty((B, N, D), np.float32)
    for c in range(NCORES):
        b, h = c // 2, c % 2
        # device layout [p, chunk, d] -> tokens (chunk*128+p, d)
        ob = res.results[c]["out"].astype(np.float32)
        out[b, h * NT : (h + 1) * NT] = ob.transpose(1, 0, 2).reshape(NT, D)
    return out
